# revision 20
# baseline (speedup 1.0000x reference)
"""2D Haar DWT (level 1) Trainium2 Bass kernel.

Input  x: [16, 64, 256, 256] f32
Output y: [16, 256, 128, 128] f32, y[n, s*64+c, i, j] = Haar mix s of the
2x2 block x[n, c, 2i:2i+2, 2j:2j+2].

Sharding: pure data parallel over the batch dim — core k gets batches
[2k, 2k+2).  Measured HW exec ~172-179 us/core (vs ~209 us f32
predecessor); ~67 MB HBM traffic/core, SDMA engines ~95% busy at
~416 GB/s effective.

Per-core design (memory-bound):

Oct-row layout: a group of G=8 channel planes (2 MB f32, contiguous in
DRAM) is loaded with an SWDGE casting DMA (f32 -> bf16) so SBUF partition
p = (c*16 + row//16) holds 16 consecutive rows — a pure reshape of the
DRAM stream (16 KB contiguous runs per partition). bf16 internals give DVE
its 2x tensor_tensor mode and halve SBUF pressure; the 2e-2 harness
tolerance dwarfs the ~4e-3 bf16 error. Butterflies:
  stage 1 (vertical):  bf16 sum/diff of row pairs -> sd [128,2,8,256]
  ACT: in-place *0.5 on sd odd columns (Haar normalization; the even
       column 0.5 folds into stage 2's scalar_tensor_tensor)
  stage 2 (horizontal): stt (even*0.5 +/- odd) with bf16 sources emitting
       f32 directly into one ot tile [128,4,8,128] holding all four
       subbands (s = 2a + v), stored as ONE merged 2 MB DMA per group
       (DRAM side [128 p][4 s][4 KB contiguous runs]).

DMA ring discipline — one job per ring so prefetch is never FIFO-blocked
behind a compute-gated transfer (mixing them costs ~15%):
  gpsimd/SWDGE ring: 16 casting loads, 7-deep prefetch
  SP/HWDGE ring (nc.sync): merged stores only
  ACT ring: nothing but the 16 small muls.
Stage 2 runs SKEW=3 groups behind stage 1, so the post-load drain phase is
pure stt work (4.4 us cadence < 5.8 us store transfer) and the ACT mul
never bubbles DVE. The last pending groups store at subband-pair / quarter
granularity so the final DMA chunk is only 512 KB (shorter tail).

Engine budget/core: DVE ~110 us, ACT ~41 us, SDMA union ~161 us
(bottleneck; exec = ~9 us framework+Q7 startup + DMA window + ~3 us
epilogue). PE unused (fp32 matmul measured slower than DVE here).

Failed variants, for the record — the DMA op structure is a sharp local
optimum; every queue-shape perturbation cost 25-40 us: f32 compute paces
DVE at ~10 us/group and starves the DMA queues (~209 us); loads mixed
onto an HWDGE store ring head-of-line block it (~202 us); G=16 tiles
halve op count but choke feed granularity (~201 us); per-group pair-split
stores insert sem waits into the SP stream (~203 us); SKEW=4 ditto
(~202 us); loads split into 1 MB halves (~215 us).
"""

import sys

sys.path.insert(0, "/opt/trn_rl_repo")

import numpy as np

import concourse.bacc as bacc
import concourse.mybir as mybir
from concourse.tile import TileContext

N_CORES = 8
N_PER_CORE = 2  # batches per core
C = 64  # input channels
H = 256
W = 256
G = 8  # channels per group (2 MB loads, 16 rows/partition)
F32 = mybir.dt.float32
BF16 = mybir.dt.bfloat16


def build_nc():
    nc = bacc.Bacc("TRN2", target_bir_lowering=False, debug=False)
    x = nc.dram_tensor("x", [N_PER_CORE, C, H, W], F32, kind="ExternalInput")
    y = nc.dram_tensor("y", [N_PER_CORE, 4 * C, H // 2, W // 2], F32, kind="ExternalOutput")

    with TileContext(nc) as tc:
        with (
            tc.tile_pool(name="inpool", bufs=6) as inpool,
            tc.tile_pool(name="in32pool", bufs=1) as in32pool,
            tc.tile_pool(name="sdpool", bufs=4) as sdpool,
            tc.tile_pool(name="outpool", bufs=5) as outpool,
        ):
            groups = [(n, c0) for n in range(N_PER_CORE) for c0 in range(0, C, G)]
            SKEW = 3  # stage-2 lags stage-1 by this many groups
            pending = []  # (sdj, n, c0) awaiting stage 2

            def stage2_and_store(sdj, n, c0, split):
                # split=0: one merged 2 MB store. split=1: store each subband
                # pair as its stt finishes (drain smoothing). split=2: quarter
                # granularity (r-halves) so the final chunk is only 512 KB.
                ot = outpool.tile([128, 4 * G * 128], F32, tag="out")
                otq = ot[:].rearrange("p (a v r j) -> p a v r j", a=2, v=2, r=G)
                dst = (
                    y[n]
                    .rearrange("(s c) h j -> c s h j", s=4)[c0 : c0 + G]
                    .rearrange("c s (q r) j -> (c q) s (r j)", r=G)
                )
                osb = ot[:].rearrange("p (s f) -> p s f", s=4)
                rh = G // 2 * 128
                for a, op1 in ((0, mybir.AluOpType.add), (1, mybir.AluOpType.subtract)):
                    if split == 2:
                        for r0 in (0, G // 2):
                            nc.vector.scalar_tensor_tensor(
                                out=otq[:, a, :, r0 : r0 + G // 2],
                                in0=sdj[:, :, r0 : r0 + G // 2, :, 0],
                                scalar=0.5,
                                in1=sdj[:, :, r0 : r0 + G // 2, :, 1],
                                op0=mybir.AluOpType.mult,
                                op1=op1,
                            )
                            lo = r0 * 128
                            nc.sync.dma_start(
                                out=dst[:, 2 * a : 2 * a + 2, lo : lo + rh],
                                in_=osb[:, 2 * a : 2 * a + 2, lo : lo + rh],
                            )
                    else:
                        nc.vector.scalar_tensor_tensor(
                            out=otq[:, a],
                            in0=sdj[..., 0],
                            scalar=0.5,
                            in1=sdj[..., 1],
                            op0=mybir.AluOpType.mult,
                            op1=op1,
                        )
                        if split == 1:
                            nc.sync.dma_start(
                                out=dst[:, 2 * a : 2 * a + 2],
                                in_=osb[:, 2 * a : 2 * a + 2],
                            )
                if split == 0:
                    nc.sync.dma_start(out=dst, in_=osb)

            for gi, (n, c0) in enumerate(groups):
                # --- load: casting reshape of the 2 MB contiguous group.
                # it[p, o, w] = bf16(x[n, c0 + p//16, 16*(p%16) + o, w])
                # Group 0 loads raw f32 over the SP/HWDGE ring: SP has no
                # activation tables, so its first op issues right after the
                # start barrier, ~3.5 us before the Q7 SWDGE warmup finishes.
                src = x[n, c0 : c0 + G].rearrange("c (q o) w -> (c q) o w", o=2 * G)
                if gi == 0:
                    it = in32pool.tile([128, G * 512], F32, tag="in32")
                    eng = nc.sync
                else:
                    it = inpool.tile([128, G * 512], BF16, tag="in")
                    eng = nc.gpsimd
                eng.dma_start(
                    out=it[:].rearrange("p (o w) -> p o w", o=2 * G), in_=src
                )

                # --- stage 1 (vertical): rows 2t / 2t+1 within a partition
                itv = it[:].rearrange("p (r t w) -> p r t w", r=G, t=2)
                sd = sdpool.tile([128, G * 512], BF16, tag="sd")
                sdv = sd[:].rearrange("p (v r w) -> p v r w", v=2, r=G)
                nc.vector.tensor_add(
                    out=sdv[:, 0], in0=itv[:, :, 0, :], in1=itv[:, :, 1, :]
                )
                nc.vector.tensor_sub(
                    out=sdv[:, 1], in0=itv[:, :, 0, :], in1=itv[:, :, 1, :]
                )

                # --- Haar 0.5 normalization on the odd columns; the even
                # column 0.5 folds into stage 2's stt.
                sdj = sd[:].rearrange("p (v r j t) -> p v r j t", v=2, r=G, t=2)
                nc.scalar.mul(sdj[..., 1], sdj[..., 1], 0.5)

                # --- stage 2 + store run SKEW groups behind stage 1, so the
                # drain phase is pure stt work (4.4 us cadence < 5.8 us store
                # transfer) and the ACT mul never bubbles DVE.
                pending.append((sdj, n, c0))
                if len(pending) > SKEW:
                    stage2_and_store(*pending.pop(0), split=0)
            for k, args in enumerate(pending):
                last = k == len(pending) - 1
                stage2_and_store(*args, split=2 if last else 1)

    nc.finalize()
    return nc


_NC = None


def _get_nc():
    global _NC
    if _NC is None:
        _NC = build_nc()
    return _NC


def kernel(x: np.ndarray) -> np.ndarray:
    from concourse.bass_utils import run_bass_kernel_spmd

    x = np.ascontiguousarray(np.asarray(x), dtype=np.float32)
    assert x.shape == (16, C, H, W), x.shape

    nc = _get_nc()
    in_maps = [
        {"x": x[k * N_PER_CORE : (k + 1) * N_PER_CORE]} for k in range(N_CORES)
    ]
    res = run_bass_kernel_spmd(nc, in_maps, core_ids=list(range(N_CORES)))
    return np.concatenate([r["y"] for r in res.results], axis=0)
